# revision 1
# baseline (speedup 1.0000x reference)
"""Efficient Channel Attention kernel for 8 Trainium2 NeuronCores.

Problem (B=4, N=4096, C=1024, H=4, HD=256):
    qkv = x @ Wqkv.T                 -> q,k,v per head, [HD, N] layout
    q,k l2-normalized over N; scores = (q*temp) @ k.T   [HD, HD] per (b,h)
    attn = softmax(scores, -1); out = attn @ v; y = out @ Wproj.T + bproj + x

Sharding: core = (batch b, token-half). All channel contractions are local;
the only cross-core coupling is the token(N)-contracted quantities: the raw
Grams k^T q and the q/k squared norms, AllReduce'd (~1MB) within the core
pair sharing a batch. Device layouts are channel-major (transposed); the
host feeds x^T / W^T slices and transposes the returned y^T back.

SBUF/PSUM pool tags are reused across phases (static pool allocation):
  wgt w0-7   : Wqkv^T qk-cols -> Wqkv^T v-cols -> Wproj^T
  xs  xs0-7  : x^T stream (A1) -> x^T stream (A2) -> B scratch / y + residual
  vo  vo0-8  : v chunks -> out^T chunks
  PSUM pA-pD : q/k accum -> sumsq rows -> v accum -> spm/atp -> proj accum
  PSUM pE,pF : Gram accumulators (2 heads each) -> attn@v accum
"""

import numpy as np

B, N, C, H = 4, 4096, 1024, 4
HD = C // H          # 256
NCORES = 8
NL = N // 2          # 2048 tokens per core
KT = C // 128        # 8 channel k-tiles
NT5 = NL // 512      # 4 token super-tiles
EPS = 1e-12

_CACHE = {}


def _build():
    import concourse.mybir as mybir
    import concourse.tile as tile
    from concourse import bacc
    from concourse.masks import make_identity

    f32 = mybir.dt.float32
    f32r = mybir.dt.float32r
    AX = mybir.AxisListType.X
    ADD = mybir.AluOpType.add
    Exp = mybir.ActivationFunctionType.Exp
    Ident = mybir.ActivationFunctionType.Identity

    nc = bacc.Bacc("TRN2", target_bir_lowering=False, debug=False,
                   num_devices=NCORES)

    xT_d = nc.dram_tensor("xT", [C, NL], f32r, kind="ExternalInput").ap()
    wqkT_d = nc.dram_tensor("wqkT", [C, 2 * C], f32r, kind="ExternalInput").ap()
    wvT_d = nc.dram_tensor("wvT", [C, C], f32r, kind="ExternalInput").ap()
    wpT_d = nc.dram_tensor("wpT", [C, C], f32r, kind="ExternalInput").ap()
    bias_d = nc.dram_tensor("bias", [128, KT], f32, kind="ExternalInput").ap()
    tmpv_d = nc.dram_tensor("tmpv", [128, KT], f32, kind="ExternalInput").ap()
    xrT_d = nc.dram_tensor("xrT", [C, NL], f32r, kind="ExternalInput").ap()
    yT_d = nc.dram_tensor("yT", [C, NL], f32, kind="ExternalOutput").ap()

    with tile.TileContext(nc) as tc:
        with (
            tc.tile_pool(name="const", bufs=1) as constp,
            tc.tile_pool(name="wgt", bufs=1) as wgtp,
            tc.tile_pool(name="xs", bufs=1) as xsp,
            tc.tile_pool(name="vo", bufs=1) as vop,
            tc.tile_pool(name="wrk", bufs=1) as wrk,
            tc.tile_pool(name="ps1", bufs=1, space="PSUM") as ps1,
            tc.tile_pool(name="ps2", bufs=1, space="PSUM") as ps2,
            tc.tile_pool(name="dram", bufs=1, space="DRAM") as dramp,
        ):
            P1 = ["pA", "pB", "pC", "pD"]  # 1-bank rotating psum tags

            # ---------------- constants ----------------
            ident = constp.tile([128, 128], f32, name="ident")
            make_identity(nc, ident[:])
            bias_sb = constp.tile([128, KT], f32, name="bias_sb")
            nc.sync.dma_start(bias_sb[:], bias_d[:])
            tmpv_sb = constp.tile([128, KT], f32, name="tmpv_sb")
            nc.sync.dma_start(tmpv_sb[:], tmpv_d[:])
            ones_sb = constp.tile([128, 1], f32, name="ones_sb")
            nc.vector.memset(ones_sb[:], 1.0)

            # first token super-tile of x^T, loaded ahead of the weights
            xst0 = []
            for kt in range(KT):
                t = xsp.tile([128, 512], f32r, tag=f"xs{kt}", bufs=2,
                             name=f"xa{kt}_0")
                nc.sync.dma_start(t[:], xT_d[kt * 128:(kt + 1) * 128, 0:512])
                xst0.append(t)
            # qk weight chunks, resident through A1
            wqk = []
            for kt in range(KT):
                w = wgtp.tile([128, 2 * C], f32r, tag=f"w{kt}", name=f"wqk{kt}")
                nc.sync.dma_start(w[:], wqkT_d[kt * 128:(kt + 1) * 128, :])
                wqk.append(w)

            # Gram accumulators: stA = heads 0,1 / stB = heads 2,3
            stA = ps2.tile([128, 1024], f32, tag="pE", name="stA")
            stB = ps2.tile([128, 1024], f32, tag="pF", name="stB")

            def st_slice(h, m):
                t = stA if h < 2 else stB
                off = (h % 2) * 512 + m * 256
                return t[:, off:off + 256]

            accq = wrk.tile([128, C], f32, tag="accq", name="accq")
            acck = wrk.tile([128, C], f32, tag="acck", name="acck")

            # ---------------- phase A1: q,k + Grams + sumsq ----------------
            for n5 in range(NT5):
                if n5 == 0:
                    xst = xst0
                else:
                    xst = []
                    for kt in range(KT):
                        t = xsp.tile([128, 512], f32r, tag=f"xs{kt}", bufs=2,
                                     name=f"xa{kt}_{n5}")
                        nc.sync.dma_start(
                            t[:], xT_d[kt * 128:(kt + 1) * 128,
                                       n5 * 512:(n5 + 1) * 512])
                        xst.append(t)
                for s in range(4):
                    tidx = n5 * 4 + s
                    qp0 = ps1.tile([128, 512], f32, tag="pA", name="qp0")
                    qp1 = ps1.tile([128, 512], f32, tag="pB", name="qp1")
                    kp0 = ps1.tile([128, 512], f32, tag="pC", name="kp0")
                    kp1 = ps1.tile([128, 512], f32, tag="pD", name="kp1")
                    for kt in range(KT):
                        lhs = xst[kt][:, s * 128:(s + 1) * 128]
                        fl, ll = (kt == 0), (kt == KT - 1)
                        nc.tensor.matmul(qp0[:], lhs, wqk[kt][:, 0:512],
                                         start=fl, stop=ll)
                        nc.tensor.matmul(qp1[:], lhs, wqk[kt][:, 512:1024],
                                         start=fl, stop=ll)
                        nc.tensor.matmul(kp0[:], lhs, wqk[kt][:, 1024:1536],
                                         start=fl, stop=ll)
                        nc.tensor.matmul(kp1[:], lhs, wqk[kt][:, 1536:2048],
                                         start=fl, stop=ll)
                    qcol = wrk.tile([128, C], f32r, tag="qcol", name="qcol")
                    kcol = wrk.tile([128, C], f32r, tag="kcol", name="kcol")
                    nc.vector.tensor_copy(qcol[:, 0:512], qp0[:])
                    nc.vector.tensor_copy(qcol[:, 512:1024], qp1[:])
                    nc.vector.tensor_copy(kcol[:, 0:512], kp0[:])
                    nc.vector.tensor_copy(kcol[:, 512:1024], kp1[:])
                    sq = wrk.tile([128, C], f32, tag="sq", name="sq")
                    sk = wrk.tile([128, C], f32, tag="sk", name="sk")
                    # square from the SBUF copies so the psum banks free
                    # after a single reader (keeps PE accumulation rolling)
                    nc.scalar.square(sq[:], qcol[:].bitcast(f32))
                    nc.scalar.square(sk[:], kcol[:].bitcast(f32))
                    if tidx == 0:
                        nc.gpsimd.tensor_copy(accq[:], sq[:])
                        nc.gpsimd.tensor_copy(acck[:], sk[:])
                    else:
                        nc.gpsimd.tensor_add(accq[:], accq[:], sq[:])
                        nc.gpsimd.tensor_add(acck[:], acck[:], sk[:])
                    for h in range(H):
                        for m in range(2):
                            nc.tensor.matmul(
                                st_slice(h, m),
                                kcol[:, h * 256 + m * 128: h * 256 + (m + 1) * 128],
                                qcol[:, h * 256:(h + 1) * 256],
                                start=(tidx == 0), stop=(tidx == 15),
                                skip_group_check=True)

            # sumsq rows: [1, 512] ones-matmuls into the freed qk psum slots
            ss_ps = []
            for i, (src, lo) in enumerate([(accq, 0), (accq, 512),
                                           (acck, 0), (acck, 512)]):
                sp = ps1.tile([1, 512], f32, tag=P1[i], name=f"ss{i}")
                nc.tensor.matmul(sp[:], ones_sb[:], src[:, lo:lo + 512],
                                 start=True, stop=True)
                ss_ps.append(sp)

            # SBUF bounces for the collective input (DMA cannot read PSUM);
            # all land in slots whose previous tenants just died.
            stA_sb = wrk.tile([128, 1024], f32, tag="qcol", name="stA_sb")
            stB_sb = wrk.tile([128, 1024], f32, tag="kcol", name="stB_sb")
            nc.vector.tensor_copy(stA_sb[:], stA[:])
            nc.vector.tensor_copy(stB_sb[:], stB[:])
            ss_sb = []
            for i, tg in enumerate(["sq", "sk", "accq", "acck"]):
                sb = wrk.tile([1, 512], f32, tag=tg, name=f"ssb{i}")
                nc.vector.tensor_copy(sb[:], ss_ps[i][:])
                ss_sb.append(sb)

            # ---------------- AllReduce over batch-pairs ----------------
            CCN = 128 * 2048 + 2 * C
            cc_in = dramp.tile([CCN], f32, name="cc_in")
            cc_out = dramp.tile([CCN], f32, name="cc_out")
            nc.sync.dma_start(
                cc_in[0:131072].rearrange("(p f) -> p f", p=128), stA_sb[:])
            nc.sync.dma_start(
                cc_in[131072:262144].rearrange("(p f) -> p f", p=128), stB_sb[:])
            for i in range(4):
                nc.sync.dma_start(
                    cc_in[262144 + i * 512: 262144 + (i + 1) * 512]
                    .rearrange("(a f) -> a f", a=1), ss_sb[i][:])
            nc.gpsimd.collective_compute(
                "AllReduce", ADD,
                replica_groups=[[0, 1], [2, 3], [4, 5], [6, 7]],
                ins=[cc_in.opt()], outs=[cc_out.opt()])
            strA = wrk.tile([128, 1024], f32, tag="qcol", name="strA")
            strB = wrk.tile([128, 1024], f32, tag="kcol", name="strB")
            nc.sync.dma_start(
                strA[:], cc_out[0:131072].rearrange("(p f) -> p f", p=128))
            nc.sync.dma_start(
                strB[:], cc_out[131072:262144].rearrange("(p f) -> p f", p=128))
            ssred = constp.tile([128, 16], f32, name="ssred")
            nc.sync.dma_start(
                ssred[:],
                cc_out[262144:262144 + 2048].rearrange("(j p) -> p j", p=128))

            def str_slice(h, m):
                t = strA if h < 2 else strB
                off = (h % 2) * 512 + m * 256
                return t[:, off:off + 256]

            # ---------------- phase A2: v (overlaps the collective) -------
            wv = []
            for kt in range(KT):
                w = wgtp.tile([128, C], f32r, tag=f"w{kt}", name=f"wv{kt}")
                nc.sync.dma_start(w[:], wvT_d[kt * 128:(kt + 1) * 128, :])
                wv.append(w)
            v_sb = [vop.tile([128, NL], f32r, tag=f"vo{cv}", name=f"v{cv}")
                    for cv in range(8)]
            pcnt = 0
            for pb in range(2):
                xst = []
                for kt in range(KT):
                    ta = xsp.tile([128, 512], f32r, tag=f"xs{kt}", bufs=2,
                                  name=f"xva{kt}_{pb}")
                    tb = xsp.tile([128, 512], f32r, tag=f"xs{kt}", bufs=2,
                                  name=f"xvb{kt}_{pb}")
                    nc.sync.dma_start(
                        ta[:], xT_d[kt * 128:(kt + 1) * 128,
                                    pb * 1024: pb * 1024 + 512])
                    nc.sync.dma_start(
                        tb[:], xT_d[kt * 128:(kt + 1) * 128,
                                    pb * 1024 + 512: pb * 1024 + 1024])
                    xst.append((ta, tb))
                for cv in range(8):
                    va = ps1.tile([128, 512], f32, tag=P1[pcnt % 4], name="vpa")
                    pcnt += 1
                    vb = ps1.tile([128, 512], f32, tag=P1[pcnt % 4], name="vpb")
                    pcnt += 1
                    for kt in range(KT):
                        fl, ll = (kt == 0), (kt == KT - 1)
                        nc.tensor.matmul(va[:],
                                         wv[kt][:, cv * 128:(cv + 1) * 128],
                                         xst[kt][0][:], start=fl, stop=ll)
                        nc.tensor.matmul(vb[:],
                                         wv[kt][:, cv * 128:(cv + 1) * 128],
                                         xst[kt][1][:], start=fl, stop=ll)
                    nc.vector.tensor_copy(
                        v_sb[cv][:, pb * 1024: pb * 1024 + 512], va[:])
                    nc.vector.tensor_copy(
                        v_sb[cv][:, pb * 1024 + 512: pb * 1024 + 1024], vb[:])

            # ---------------- phase B: normalize + softmax + attn@v -------
            # rq = temp/max(sqrt(ssq),eps), rk = 1/max(sqrt(ssk),eps), as
            # per-partition columns [128, 16]: cols 0-7 = rq, 8-15 = rk.
            rqk = constp.tile([128, 16], f32, name="rqk")
            nc.scalar.sqrt(rqk[:], ssred[:])
            nc.vector.tensor_scalar_max(rqk[:], rqk[:], EPS)
            nc.vector.reciprocal(rqk[:], rqk[:])
            nc.vector.tensor_mul(rqk[:, 0:8], rqk[:, 0:8], tmpv_sb[:])

            outT = []
            for h in range(H):
                # Gram^T rows d scaled by rk[d]
                sth = xsp.tile([128, 512], f32, tag="xs4", bufs=2, name="sth")
                for m in range(2):
                    nc.vector.tensor_scalar_mul(
                        sth[:, m * 256:(m + 1) * 256], str_slice(h, m),
                        rqk[:, 8 + 2 * h + m: 9 + 2 * h + m])
                # transpose to S[c, d]
                spm = ps1.tile([128, 512], f32, tag="pA", name="spm")
                for mc in range(2):
                    for md in range(2):
                        nc.tensor.transpose(
                            spm[:, mc * 256 + md * 128: mc * 256 + (md + 1) * 128],
                            sth[:, md * 256 + mc * 128: md * 256 + (mc + 1) * 128],
                            ident[:])
                sft = xsp.tile([128, 512], f32, tag="xs5", bufs=2, name="sft")
                for mc in range(2):
                    nc.vector.tensor_scalar_mul(
                        sft[:, mc * 256:(mc + 1) * 256],
                        spm[:, mc * 256:(mc + 1) * 256],
                        rqk[:, 2 * h + mc: 1 + 2 * h + mc])
                # softmax over d (free axis)
                negmax = wrk.tile([128, 2], f32, tag="negmax", name="negmax")
                rowsum = wrk.tile([128, 2], f32, tag="rowsum", name="rowsum")
                recip = wrk.tile([128, 2], f32, tag="recip", name="recip")
                esb = xsp.tile([128, 512], f32, tag="xs6", bufs=2, name="esb")
                for mc in range(2):
                    nc.vector.reduce_max(negmax[:, mc:mc + 1],
                                         sft[:, mc * 256:(mc + 1) * 256],
                                         axis=AX, negate=True)
                    nc.scalar.activation(esb[:, mc * 256:(mc + 1) * 256],
                                         sft[:, mc * 256:(mc + 1) * 256],
                                         Exp, bias=negmax[:, mc:mc + 1],
                                         accum_out=rowsum[:, mc:mc + 1])
                nc.vector.reciprocal(recip[:], rowsum[:])
                # attn^T (columns d on partitions)
                atp = ps1.tile([128, 512], f32, tag="pB", name="atp")
                for md in range(2):
                    for mc in range(2):
                        nc.tensor.transpose(
                            atp[:, md * 256 + mc * 128: md * 256 + (mc + 1) * 128],
                            esb[:, mc * 256 + md * 128: mc * 256 + (md + 1) * 128],
                            ident[:])
                atn = xsp.tile([128, 512], f32r, tag="xs7", bufs=2, name="atn")
                nc.vector.tensor_copy(atn[:], atp[:])
                # out^T[c,:] = sum_d attn^T[d,c] v[d,:], row-scaled by 1/rowsum
                op2s = {}
                for mc in range(2):
                    for nfh in range(2):
                        op2 = ps2.tile([128, 1024], f32,
                                       tag=("pE" if nfh == 0 else "pF"),
                                       name="op2")
                        op2s[(mc, nfh)] = op2
                        for md in range(2):
                            for n2 in range(2):
                                nc.tensor.matmul(
                                    op2[:, n2 * 512:(n2 + 1) * 512],
                                    atn[:, md * 256 + mc * 128: md * 256 + (mc + 1) * 128],
                                    v_sb[2 * h + md][:, nfh * 1024 + n2 * 512:
                                                     nfh * 1024 + (n2 + 1) * 512],
                                    start=(md == 0), stop=(md == 1))
                for mc in range(2):
                    i = 2 * h + mc
                    ot = vop.tile([128, NL], f32r, tag=f"vo{(i + 8) % 9}",
                                  name=f"ot{i}")
                    outT.append(ot)
                    for nfh in range(2):
                        nc.vector.tensor_scalar_mul(
                            ot[:, nfh * 1024:(nfh + 1) * 1024],
                            op2s[(mc, nfh)][:], recip[:, mc:mc + 1])

            # ---------------- phase C: projection + bias + residual -------
            wp = []
            for kt in range(KT):
                w = wgtp.tile([128, C], f32r, tag=f"w{kt}", name=f"wp{kt}")
                nc.sync.dma_start(w[:], wpT_d[kt * 128:(kt + 1) * 128, :])
                wp.append(w)
            for j in range(KT):
                pq = []
                for q in range(4):
                    p = ps1.tile([128, 512], f32, tag=P1[q], name=f"pp{q}")
                    pq.append(p)
                for kt in range(KT):
                    # proj input channel chunk kt = (jj=kt//2, d-half=kt%2);
                    # column block q is head q; tokens subsampled jj::4
                    for q in range(4):
                        nc.tensor.matmul(
                            pq[q][:],
                            wp[kt][:, j * 128:(j + 1) * 128],
                            outT[2 * q + kt % 2][:, (kt // 2)::4],
                            start=(kt == 0), stop=(kt == KT - 1))
                for q in range(4):
                    xr = xsp.tile([128, 512], f32r, tag=f"xs{4 + q}", bufs=2,
                                  name=f"xr{j}_{q}")
                    nc.sync.dma_start(
                        xr[:], xrT_d[j * 128:(j + 1) * 128,
                                     q * 512:(q + 1) * 512])
                    yq = xsp.tile([128, 512], f32, tag=f"xs{q}", bufs=2,
                                  name=f"yq{j}_{q}")
                    nc.scalar.activation(yq[:], pq[q][:], Ident,
                                         bias=bias_sb[:, j:j + 1])
                    nc.vector.tensor_add(yq[:], yq[:], xr[:].bitcast(f32))
                    nc.sync.dma_start(
                        yT_d[j * 128:(j + 1) * 128, q * 512:(q + 1) * 512],
                        yq[:])

    nc.compile()
    return nc


def _get_nc():
    if "nc" not in _CACHE:
        _CACHE["nc"] = _build()
    return _CACHE["nc"]


def _make_in_maps(x, Wqkv, Wproj, bproj, temperature):
    x = np.ascontiguousarray(np.asarray(x, dtype=np.float32))
    Wqkv = np.asarray(Wqkv, dtype=np.float32)
    Wproj = np.asarray(Wproj, dtype=np.float32)
    bproj = np.asarray(bproj, dtype=np.float32).reshape(C)
    temp = np.asarray(temperature, dtype=np.float32).reshape(H)

    WqkvT = np.ascontiguousarray(Wqkv.T)          # [C, 3C]
    wqkT = np.ascontiguousarray(WqkvT[:, :2 * C])
    wvT = np.ascontiguousarray(WqkvT[:, 2 * C:])
    wpT = np.ascontiguousarray(Wproj.T)
    bias2d = np.ascontiguousarray(bproj.reshape(KT, 128).T)
    tmpv2d = np.ascontiguousarray(np.repeat(temp, HD).reshape(KT, 128).T)

    in_maps = []
    for core in range(NCORES):
        b, half = core // 2, core % 2
        xT = np.ascontiguousarray(x[b, half * NL:(half + 1) * NL, :].T)
        rows = _out_rows(half)
        xrT = np.ascontiguousarray(x[b, rows, :].T)
        in_maps.append(dict(xT=xT, xrT=xrT, wqkT=wqkT, wvT=wvT, wpT=wpT,
                            bias=bias2d, tmpv=tmpv2d))
    return in_maps


def _out_rows(half):
    # torch transpose+reshape scramble: this core's y rows
    return np.concatenate(
        [h * 1024 + half * 512 + np.arange(512) for h in range(H)])


def _run(in_maps, trace=False, **kw):
    from concourse.bass_utils import run_bass_kernel_spmd

    nc = _get_nc()
    return run_bass_kernel_spmd(nc, in_maps, core_ids=list(range(NCORES)),
                                trace=trace, **kw)


def kernel(x, Wqkv, Wproj, bproj, temperature):
    res = _run(_make_in_maps(x, Wqkv, Wproj, bproj, temperature))
    y = np.empty((B, N, C), dtype=np.float32)
    for core in range(NCORES):
        b, half = core // 2, core % 2
        y[b, _out_rows(half), :] = res.results[core]["yT"].T
    return y



# revision 3
# speedup vs baseline: 1.9308x; 1.9308x over previous
"""Efficient Channel Attention kernel for 8 Trainium2 NeuronCores.

Problem (B=4, N=4096, C=1024, H=4, HD=256):
    qkv = x @ Wqkv.T; q,k l2-normalized over N; scores = (q*temp) @ k.T
    attn = softmax(scores, -1); out = attn @ v; y = out @ Wproj.T + bproj + x

Sharding: core = (batch b, head-pair hp). Each core computes heads
{2hp, 2hp+1} over ALL 4096 tokens of its batch, so the token-contracted
Grams and q/k norms are complete locally — NO collective at all. The
output rows owned by head h are y[h*1024:(h+1)*1024], so each core owns
the contiguous y rows [2048*hp, 2048*hp+2048).

Token permutation: on-chip local token index l = m*1024 + j' maps to
global token n = 4*j' + m (the torch transpose+reshape scramble). With
this order v/outT are naturally m-blocked and every matmul operand is
contiguous (no strided rhs — 8x penalty measured on HW).

Dtypes: the three big GEMMs (qkv-projection, v-projection, proj) run in
fp8e4m3 with DoubleRow perf mode (K=256 per pass, 2x throughput);
weights are host-scaled x16 so fp8 operands sit in the normal range,
and the 1/256 net scale is folded into the output activation. Grams and
attn@v run in bf16; norms/softmax/residual in fp32.
"""

import numpy as np

B, N, C, H = 4, 4096, 1024, 4
HD = C // H          # 256
NCORES = 8
NT = 32              # 128-token sub-tiles
SW = 16.0            # host weight scale for fp8 range
EPS = 1e-12

_CACHE = {}


def _build():
    import concourse.mybir as mybir
    import concourse.tile as tile
    from concourse import bacc
    from concourse.masks import make_identity

    f32 = mybir.dt.float32
    bf16 = mybir.dt.bfloat16
    fp8 = mybir.dt.float8e4
    DR = mybir.MatmulPerfMode.DoubleRow
    AX = mybir.AxisListType.X
    Exp = mybir.ActivationFunctionType.Exp
    Ident = mybir.ActivationFunctionType.Identity

    nc = bacc.Bacc("TRN2", target_bir_lowering=False, debug=False,
                   num_devices=NCORES)

    # paired-ktile layouts: row 128*j+p, free (i, n) holds src[256j+128i+p, n]
    xT_d = nc.dram_tensor("xT", [512, 2 * N], fp8, kind="ExternalInput").ap()
    wqk_d = nc.dram_tensor("wqk", [512, 2048], fp8, kind="ExternalInput").ap()
    wv_d = nc.dram_tensor("wv", [512, 1024], fp8, kind="ExternalInput").ap()
    wp_d = nc.dram_tensor("wp", [512, 2048], fp8, kind="ExternalInput").ap()
    xrT_d = nc.dram_tensor("xrT", [C, 2048], f32, kind="ExternalInput").ap()
    bias_d = nc.dram_tensor("bias", [128, 8], f32, kind="ExternalInput").ap()
    tmpv_d = nc.dram_tensor("tmpv", [128, 4], f32, kind="ExternalInput").ap()
    yT_d = nc.dram_tensor("yT", [C, 2048], f32, kind="ExternalOutput").ap()

    with tile.TileContext(nc) as tc:
        with (
            tc.tile_pool(name="const", bufs=1) as constp,
            tc.tile_pool(name="wgt", bufs=1) as wgtp,
            tc.tile_pool(name="xs", bufs=1) as xsp,
            tc.tile_pool(name="vo", bufs=1) as vop,
            tc.tile_pool(name="wrk", bufs=1) as wrk,
            tc.tile_pool(name="ps", bufs=1, space="PSUM") as ps,
        ):
            # ---------------- constants + weights ----------------
            ident = constp.tile([128, 128], f32, name="ident")
            make_identity(nc, ident[:])
            bias_sb = constp.tile([128, 8], f32, name="bias_sb")
            nc.sync.dma_start(bias_sb[:], bias_d[:])
            tmpv_sb = constp.tile([128, 4], f32, name="tmpv_sb")
            nc.sync.dma_start(tmpv_sb[:], tmpv_d[:])

            # x resident, 4 paired-ktile tiles [128, 2, 4096] fp8;
            # quarter-granular DMA so the first matmuls start early
            xst = []
            for jp in range(4):
                t = xsp.tile([128, 2, N], fp8, name=f"x{jp}")
                src = xT_d[jp * 128:(jp + 1) * 128, :].rearrange(
                    "p (two f) -> p two f", two=2)
                for q in range(4):
                    nc.sync.dma_start(
                        t[:, :, q * 1024:(q + 1) * 1024],
                        src[:, :, q * 1024:(q + 1) * 1024])
                xst.append(t)
            wqk = []
            for jp in range(4):
                w = wgtp.tile([128, 2, 1024], fp8, name=f"wqk{jp}")
                nc.sync.dma_start(
                    w[:], wqk_d[jp * 128:(jp + 1) * 128, :].rearrange(
                        "p (two f) -> p two f", two=2))
                wqk.append(w)
            wv = []
            for jp in range(4):
                w = wgtp.tile([128, 2, 512], fp8, name=f"wv{jp}")
                nc.sync.dma_start(
                    w[:], wv_d[jp * 128:(jp + 1) * 128, :].rearrange(
                        "p (two f) -> p two f", two=2))
                wv.append(w)
            wp = []
            for m in range(4):
                w = wgtp.tile([128, 2, 1024], fp8, name=f"wp{m}")
                nc.sync.dma_start(
                    w[:], wp_d[m * 128:(m + 1) * 128, :].rearrange(
                        "p (two f) -> p two f", two=2))
                wp.append(w)

            v_sb = [vop.tile([128, N], bf16, name=f"v{cv}") for cv in range(4)]
            accq = wrk.tile([128, 512], f32, name="accq")
            acck = wrk.tile([128, 512], f32, name="acck")
            stG = [ps.tile([128, 512], f32, tag=f"pg{hh}", name=f"stG{hh}")
                   for hh in range(2)]

            # ---------------- A1: q,k + Grams + sumsq ----------------
            for t in range(NT):
                qp = ps.tile([128, 512], f32, tag=f"pq{t % 2}", name="qp")
                kp = ps.tile([128, 512], f32, tag=f"pk{t % 2}", name="kp")
                for jp in range(4):
                    lhs = xst[jp][:, :, t * 128:(t + 1) * 128]
                    nc.tensor.matmul(qp[:], lhs, wqk[jp][:, :, 0:512],
                                     start=(jp == 0), stop=(jp == 3),
                                     perf_mode=DR)
                for jp in range(4):
                    lhs = xst[jp][:, :, t * 128:(t + 1) * 128]
                    nc.tensor.matmul(kp[:], lhs, wqk[jp][:, :, 512:1024],
                                     start=(jp == 0), stop=(jp == 3),
                                     perf_mode=DR)
                qcol = wrk.tile([128, 512], bf16, tag="qcol", bufs=3,
                                name="qcol")
                kcol = wrk.tile([128, 512], bf16, tag="kcol", bufs=3,
                                name="kcol")
                nc.vector.tensor_copy(qcol[:], qp[:])
                nc.vector.tensor_copy(kcol[:], kp[:])
                sq = wrk.tile([128, 512], f32, tag="sq", bufs=2, name="sq")
                sk = wrk.tile([128, 512], f32, tag="sk", bufs=2, name="sk")
                nc.scalar.square(sq[:], qp[:])
                nc.scalar.square(sk[:], kp[:])
                if t == 0:
                    nc.gpsimd.tensor_copy(accq[:], sq[:])
                    nc.gpsimd.tensor_copy(acck[:], sk[:])
                else:
                    nc.gpsimd.tensor_add(accq[:], accq[:], sq[:])
                    nc.gpsimd.tensor_add(acck[:], acck[:], sk[:])
                for hh in range(2):
                    for m in range(2):
                        nc.tensor.matmul(
                            stG[hh][:, m * 256:(m + 1) * 256],
                            kcol[:, hh * 256 + m * 128:
                                 hh * 256 + (m + 1) * 128],
                            qcol[:, hh * 256:(hh + 1) * 256],
                            start=(t == 0), stop=(t == NT - 1),
                            skip_group_check=True)

            # ---------------- norms -> rqk [128, 8] ----------------
            # transpose accq/acck 128-chunks, reduce over old partitions
            nsp = ps.tile([128, 512], f32, tag="pq0", name="nsp")
            nsk = ps.tile([128, 512], f32, tag="pq1", name="nsk")
            for ch in range(4):
                nc.tensor.transpose(nsp[:, ch * 128:(ch + 1) * 128],
                                    accq[:, ch * 128:(ch + 1) * 128], ident[:])
                nc.tensor.transpose(nsk[:, ch * 128:(ch + 1) * 128],
                                    acck[:, ch * 128:(ch + 1) * 128], ident[:])
            rqk = constp.tile([128, 8], f32, name="rqk")
            for ch in range(4):
                nc.vector.reduce_sum(rqk[:, ch:ch + 1],
                                     nsp[:, ch * 128:(ch + 1) * 128], axis=AX)
                nc.vector.reduce_sum(rqk[:, 4 + ch:5 + ch],
                                     nsk[:, ch * 128:(ch + 1) * 128], axis=AX)
            nc.scalar.sqrt(rqk[:], rqk[:])
            nc.vector.tensor_scalar_max(rqk[:], rqk[:], EPS)
            nc.vector.reciprocal(rqk[:], rqk[:])
            nc.vector.tensor_mul(rqk[:, 0:4], rqk[:, 0:4], tmpv_sb[:])

            # ---------------- A2: v (overlaps B_h0 softmax prep) -------
            for cv in range(4):
                for st in range(8):
                    vp = ps.tile([128, 512], f32, tag=f"pv{st % 2}",
                                 name="vp")
                    for jp in range(4):
                        nc.tensor.matmul(
                            vp[:], wv[jp][:, :, cv * 128:(cv + 1) * 128],
                            xst[jp][:, :, st * 512:(st + 1) * 512],
                            start=(jp == 0), stop=(jp == 3), perf_mode=DR)
                    nc.vector.tensor_copy(
                        v_sb[cv][:, st * 512:(st + 1) * 512], vp[:])

            # ---------------- B + C per head ----------------
            OT = [[None] * 4, [None] * 4]
            for hh in range(2):
                # scores^T rows d scaled by rk[d]
                sth = wrk.tile([128, 512], f32, tag="sth", bufs=2, name="sth")
                for m in range(2):
                    nc.vector.tensor_scalar_mul(
                        sth[:, m * 256:(m + 1) * 256],
                        stG[hh][:, m * 256:(m + 1) * 256],
                        rqk[:, 4 + 2 * hh + m:5 + 2 * hh + m])
                spm = ps.tile([128, 512], f32, tag="pk0", name="spm")
                for mc in range(2):
                    for md in range(2):
                        nc.tensor.transpose(
                            spm[:, mc * 256 + md * 128:
                                mc * 256 + (md + 1) * 128],
                            sth[:, md * 256 + mc * 128:
                                md * 256 + (mc + 1) * 128],
                            ident[:])
                sft = wrk.tile([128, 512], f32, tag="sft", bufs=2, name="sft")
                for mc in range(2):
                    nc.vector.tensor_scalar_mul(
                        sft[:, mc * 256:(mc + 1) * 256],
                        spm[:, mc * 256:(mc + 1) * 256],
                        rqk[:, 2 * hh + mc:1 + 2 * hh + mc])
                negmax = wrk.tile([128, 2], f32, tag="negmax", bufs=2,
                                  name="negmax")
                rowsum = wrk.tile([128, 2], f32, tag="rowsum", bufs=2,
                                  name="rowsum")
                recip = wrk.tile([128, 2], f32, tag="recip", bufs=2,
                                 name="recip")
                esb = wrk.tile([128, 512], f32, tag="esb", bufs=2, name="esb")
                for mc in range(2):
                    nc.vector.reduce_max(negmax[:, mc:mc + 1],
                                         sft[:, mc * 256:(mc + 1) * 256],
                                         axis=AX, negate=True)
                    nc.scalar.activation(esb[:, mc * 256:(mc + 1) * 256],
                                         sft[:, mc * 256:(mc + 1) * 256],
                                         Exp, bias=negmax[:, mc:mc + 1],
                                         accum_out=rowsum[:, mc:mc + 1])
                nc.vector.reciprocal(recip[:], rowsum[:])
                atp = ps.tile([128, 512], f32, tag="pk1", name="atp")
                for md in range(2):
                    for mc in range(2):
                        nc.tensor.transpose(
                            atp[:, md * 256 + mc * 128:
                                md * 256 + (mc + 1) * 128],
                            esb[:, mc * 256 + md * 128:
                                mc * 256 + (md + 1) * 128],
                            ident[:])
                atn = wrk.tile([128, 512], bf16, tag="atn", bufs=2,
                               name="atn")
                nc.vector.tensor_copy(atn[:], atp[:])
                # attn @ v -> outT tiles [128, 2, 1024] fp8 (DR pairs = mc)
                OVT = ["pv0", "pv1", "pq0"]
                pcnt = 0
                for m in range(4):
                    ot = vop.tile([128, 2, 1024], fp8, tag=f"ot{m}", bufs=2,
                                  name=f"ot{hh}_{m}")
                    OT[hh][m] = ot
                    for mc in range(2):
                        for jh in range(2):
                            op = ps.tile([128, 512], f32,
                                         tag=OVT[pcnt % 3], name="op")
                            pcnt += 1
                            for md in range(2):
                                nc.tensor.matmul(
                                    op[:],
                                    atn[:, md * 256 + mc * 128:
                                        md * 256 + (mc + 1) * 128],
                                    v_sb[2 * hh + md][
                                        :, m * 1024 + jh * 512:
                                        m * 1024 + (jh + 1) * 512],
                                    start=(md == 0), stop=(md == 1))
                            nc.vector.tensor_scalar_mul(
                                ot[:, mc, jh * 512:(jh + 1) * 512], op[:],
                                recip[:, mc:mc + 1])

                # ---------------- C: proj + bias + residual ----------
                PJT = ["pq1", "pg0"]
                for j in range(8):
                    xr = xsp.tile([128, 1024], f32, tag="xr", bufs=2,
                                  name=f"xr{hh}_{j}")
                    nc.sync.dma_start(
                        xr[:], xrT_d[j * 128:(j + 1) * 128,
                                     hh * 1024:(hh + 1) * 1024])
                    for jh in range(2):
                        pp = ps.tile([128, 512], f32, tag=PJT[(j * 2 + jh) % 2],
                                     name="pp")
                        for m in range(4):
                            nc.tensor.matmul(
                                pp[:], wp[m][:, :, j * 128:(j + 1) * 128],
                                OT[hh][m][:, :, jh * 512:(jh + 1) * 512],
                                start=(m == 0), stop=(m == 3), perf_mode=DR)
                        yq = xsp.tile([128, 512], f32, tag="yq", bufs=3,
                                      name=f"yq{hh}_{j}_{jh}")
                        nc.scalar.activation(yq[:], pp[:], Ident,
                                             bias=bias_sb[:, j:j + 1],
                                             scale=1.0 / (SW * SW))
                        nc.vector.tensor_add(
                            yq[:], yq[:], xr[:, jh * 512:(jh + 1) * 512])
                        nc.sync.dma_start(
                            yT_d[j * 128:(j + 1) * 128,
                                 hh * 1024 + jh * 512:
                                 hh * 1024 + (jh + 1) * 512],
                            yq[:])

    nc.compile()
    return nc


def _get_nc():
    if "nc" not in _CACHE:
        _CACHE["nc"] = _build()
    return _CACHE["nc"]


def _drpack(a):
    """[1024, F] -> [512, 2F]: row 128j+p, free (i, f) = a[256j+128i+p, f]."""
    f = a.shape[1]
    return np.ascontiguousarray(
        a.reshape(4, 2, 128, f).transpose(0, 2, 1, 3).reshape(512, 2 * f))


def _make_in_maps(x, Wqkv, Wproj, bproj, temperature):
    import ml_dtypes

    fp8 = ml_dtypes.float8_e4m3
    x = np.ascontiguousarray(np.asarray(x, dtype=np.float32))
    Wqkv = np.asarray(Wqkv, dtype=np.float32)
    Wproj = np.asarray(Wproj, dtype=np.float32)
    bproj = np.asarray(bproj, dtype=np.float32).reshape(C)
    temp = np.asarray(temperature, dtype=np.float32).reshape(H)

    WqkvT = Wqkv.T  # [C, 3C]
    # token permutation: local l = m*1024 + j'  ->  global n = 4j' + m
    ell = np.arange(N)
    perm = 4 * (ell % 1024) + ell // 1024

    # per-batch fp8 x in paired-ktile layout
    xdr = []
    for b in range(B):
        xp = np.ascontiguousarray(x[b].T[:, perm])
        xdr.append(_drpack(xp).astype(fp8))

    # per-head-pair weights
    wqk_hp, wv_hp = [], []
    for hp in range(2):
        h0, h1 = 2 * hp, 2 * hp + 1
        qk = np.concatenate([
            WqkvT[:, 256 * h0:256 * h0 + 256],
            WqkvT[:, 256 * h1:256 * h1 + 256],
            WqkvT[:, 1024 + 256 * h0:1024 + 256 * h0 + 256],
            WqkvT[:, 1024 + 256 * h1:1024 + 256 * h1 + 256]], axis=1)
        vv = np.concatenate([
            WqkvT[:, 2048 + 256 * h0:2048 + 256 * h0 + 256],
            WqkvT[:, 2048 + 256 * h1:2048 + 256 * h1 + 256]], axis=1)
        wqk_hp.append(_drpack(qk * SW).astype(fp8))
        wv_hp.append(_drpack(vv * SW).astype(fp8))
    wp8 = _drpack(Wproj.T * SW).astype(fp8)
    bias2d = np.ascontiguousarray(bproj.reshape(8, 128).T)

    in_maps = []
    for core in range(NCORES):
        b, hp = core // 2, core % 2
        h0, h1 = 2 * hp, 2 * hp + 1
        xrT = np.ascontiguousarray(
            x[b, 2048 * hp:2048 * hp + 2048, :].T)
        tmpv = np.broadcast_to(
            np.array([temp[h0], temp[h0], temp[h1], temp[h1]],
                     dtype=np.float32), (128, 4)).copy()
        in_maps.append(dict(xT=xdr[b], wqk=wqk_hp[hp], wv=wv_hp[hp],
                            wp=wp8, xrT=xrT, bias=bias2d, tmpv=tmpv))
    return in_maps


def _run(in_maps, trace=False, **kw):
    from concourse.bass_utils import run_bass_kernel_spmd

    nc = _get_nc()
    return run_bass_kernel_spmd(nc, in_maps, core_ids=list(range(NCORES)),
                                trace=trace, **kw)


def kernel(x, Wqkv, Wproj, bproj, temperature):
    res = _run(_make_in_maps(x, Wqkv, Wproj, bproj, temperature))
    y = np.empty((B, N, C), dtype=np.float32)
    for core in range(NCORES):
        b, hp = core // 2, core % 2
        y[b, 2048 * hp:2048 * hp + 2048, :] = res.results[core]["yT"].T
    return y
